# revision 5
# baseline (speedup 1.0000x reference)
"""DangoCutouts Trainium2 kernel.

Computes reference:
    out[16, 3, 512, 512] =
      [full, gray(full), flip(full), gray(flip(full)), inner_0..11]
    where full = bilinear_resize(img, 4096 -> 512),
          inner_k = bilinear_resize(img[offy_k:+s_k, offx_k:+s_k] -> 512),
          inner_0 additionally grayscaled.

Strategy (8 NeuronCores, data-parallel over output rows):
  Core c computes output rows [64c, 64c+64) of all 16 outputs.
  13 distinct resamples (full + 12 inner). Per resample, per core:
    1. Row gather (dma_gather, SWDGE): T[128, 3, W] where
       partition p = (c2, i): c2 in {ch0, ch1}, i = strip row.
       free q-slots: q0 = y0-row, q1 = y1-row of channel c2;
       q2 = ch2 rows (p<64: y0, p>=64: y1).
       The overview (full) resample uses affine HWDGE DMAs instead
       (rows 8i+3 / 8i+4 are affine).
    2. Row combine (DVE): R01[128, W] = T[:,0]*(1-wy) + T[:,1]*wy;
       ch2 via cross-partition copy then same combine -> R2[64, W].
    3. Column gather (gpsimd ap_gather): G = R[:, xidx] for x0 and x1.
    4. Column combine (DVE): O = G0*(1-wx) + G1*wx, wx broadcast to all
       partitions via a K=1 PE matmul into PSUM.
    5. gray / flip variants derived on-chip (flip = gather with reversed
       x index tables; gray = weighted channel sum).

All index/weight tables are computed on host in float32 exactly as the
reference does and passed as per-core runtime inputs; the compiled program
structure depends only on the per-resample column windows (cached).
"""
import os
import numpy as np

CUT = 512
H = W = 4096
GRAY_W = (0.2989, 0.587, 0.114)
N_INNER = 12
NSPEC = 13          # full + 12 inner
STRIP = 64          # output rows per core
NCORES = 8
CHUNK = 1024        # row-gather column chunk (elements)
SINGLE_PACKET = True

_CACHE = {}


# --------------------------------------------------------------------------
# host-side parameter math (replicates reference._crop_resize in float32)
# --------------------------------------------------------------------------

def _bilinear_params(offy, offx, size):
    s = np.float32(size)
    t = (np.arange(CUT, dtype=np.float32) + np.float32(0.5)) * s / np.float32(CUT) \
        - np.float32(0.5)
    y = np.clip(np.float32(offy) + t, np.float32(offy), np.float32(offy) + s - np.float32(1.0))
    x = np.clip(np.float32(offx) + t, np.float32(offx), np.float32(offx) + s - np.float32(1.0))
    y0 = np.floor(y).astype(np.int32)
    x0 = np.floor(x).astype(np.int32)
    y1 = np.minimum(y0 + 1, np.int32(offy) + np.int32(size) - 1)
    x1 = np.minimum(x0 + 1, np.int32(offx) + np.int32(size) - 1)
    wy = (y - y0.astype(np.float32)).astype(np.float32)
    wx = (x - x0.astype(np.float32)).astype(np.float32)
    # match XLA gather out-of-bounds clamp / negative wrap for degenerate inputs
    for a in (y0, y1):
        np.copyto(a, np.where(a < 0, a % H, np.minimum(a, H - 1)))
    for a in (x0, x1):
        np.copyto(a, np.where(a < 0, a % W, np.minimum(a, W - 1)))
    return y0, y1, wy, x0, x1, wx


def _col_window(x0, x1):
    cx0 = int(x0[0])
    w = int(x1[-1]) - cx0 + 1
    w_al = min((w + 63) // 64 * 64, W)
    if cx0 + w_al > W:
        cx0 = W - w_al
    return cx0, w_al


def _wrap16(idx):
    """gpsimd idx-table layout: idx[i] -> [16g + i%16, i//16] for all groups g."""
    idx = np.asarray(idx, np.int16)
    n = len(idx)
    assert n % 16 == 0
    cols = n // 16
    tile = np.zeros((128, cols), np.int16)
    blk = idx.reshape(cols, 16).T
    for g in range(8):
        tile[16 * g:16 * g + 16, :] = blk
    return tile


def _specs_from_inputs(sizes, offy, offx):
    specs = [(0, 0, min(H, W))]
    for k in range(N_INNER):
        specs.append((int(offy[k]), int(offx[k]), max(int(sizes[k]), 0)))
    return specs


def _params(specs):
    out = []
    for (oy, ox, s) in specs:
        y0, y1, wy, x0, x1, wx = _bilinear_params(oy, ox, max(s, 1) if s <= 0 else s)
        cx0, w_al = _col_window(x0, x1)
        out.append(dict(y0=y0, y1=y1, wy=wy, x0=x0, x1=x1, wx=wx, cx0=cx0, w_al=w_al))
    return out


# --------------------------------------------------------------------------
# device program
# --------------------------------------------------------------------------

def _build_bass(geom, reps=1):
    """geom: tuple of (cx0, w_al) per resample. Returns (nc, names)."""
    import concourse.bacc as bacc
    import concourse.mybir as mybir
    from concourse.tile import TileContext

    f32 = mybir.dt.float32
    i16 = mybir.dt.int16
    MUL = mybir.AluOpType.mult
    ADD = mybir.AluOpType.add

    nc = bacc.Bacc("TRN2", target_bir_lowering=False, num_swdge_queues=4)

    img = nc.dram_tensor("img", [3, H, W], f32, kind="ExternalInput")
    img_rows = img.rearrange("c h w -> (c h) w")
    # per-resample row-gather idx tables, 384 idxs -> [128, 24] each
    ridx = nc.dram_tensor("ridx", [128, NSPEC * 24], i16, kind="ExternalInput")
    # col-gather idx tables: per resample x0,x1 (+ reversed x0,x1 for overview)
    # 512 idxs -> [128, 32] each; layout [NSPEC*2 + 2 tables]
    NCTAB = NSPEC * 2 + 2
    cidx = nc.dram_tensor("cidx", [128, NCTAB * 32], i16, kind="ExternalInput")
    # row weights: [128, 2*NSPEC]: cols 2k = 1-wy, 2k+1 = wy (dup across c2)
    wyt = nc.dram_tensor("wyt", [128, 2 * NSPEC], f32, kind="ExternalInput")
    # col weights (1-wx | wx interleaved per resample + reversed pair for ovw):
    # [1, (NSPEC+1)*1024]: per table r: [512 of 1-wx, 512 of wx]
    wxt = nc.dram_tensor("wxt", [NSPEC + 2, 1024], f32, kind="ExternalInput")

    out_d = nc.dram_tensor("out", [16, 3, STRIP, CUT], f32, kind="ExternalOutput")
    out_rows = out_d.rearrange("k c i j -> (k c i) j")

    def out_ap(k, c, nch=1):
        """[(nch*64), 512] AP at output image k, channel c."""
        base = (k * 3 + c) * STRIP
        return out_rows[base:base + nch * STRIP, :]

    with TileContext(nc) as tc:
        with (
            tc.tile_pool(name="const", bufs=1) as cpool,
            tc.tile_pool(name="tchunk", bufs=4) as tpool,
            tc.tile_pool(name="rslab", bufs=2) as rpool,
            tc.tile_pool(name="gtiles", bufs=2) as gpool,
            tc.tile_pool(name="otiles", bufs=2) as opool,
            tc.tile_pool(name="psum", bufs=2, space="PSUM") as ppool,
        ):
            # ---- constants ----
            ridx_t = cpool.tile([128, NSPEC * 24], i16)
            nc.sync.dma_start(out=ridx_t[:], in_=ridx[:])
            cidx_t = cpool.tile([128, NCTAB * 32], i16)
            nc.sync.dma_start(out=cidx_t[:], in_=cidx[:])
            wyt_t = cpool.tile([128, 2 * NSPEC], f32)
            nc.sync.dma_start(out=wyt_t[:], in_=wyt[:])
            ones_t = cpool.tile([1, 128], f32)
            nc.sync.dma_start(out=ones_t[:], in_=wxt[NSPEC + 1:NSPEC + 2, 0:128])

            def wx_bcast(tab):
                """[128, 1024] PSUM tile: cols 0:512 = 1-wx, 512: = wx."""
                wx_t = gpool.tile([1, 1024], f32, tag="wxrow")
                nc.sync.dma_start(out=wx_t[:], in_=wxt[tab:tab + 1, :])
                p = ppool.tile([128, 1024], f32, space="PSUM")
                nc.tensor.matmul(out=p[:, 0:CUT], lhsT=ones_t[:], rhs=wx_t[:, 0:CUT],
                                 start=True, stop=True)
                nc.tensor.matmul(out=p[:, CUT:], lhsT=ones_t[:], rhs=wx_t[:, CUT:],
                                 start=True, stop=True)
                ps = gpool.tile([128, 1024], f32, tag="wxb")
                nc.scalar.copy(out=ps[:], in_=p[:])
                return ps

            def col_stage(R01, R2, w_al, ctab, wxp, kout, gray=False, gray_only=False):
                """Gather columns + combine + write one output image (and
                optionally its gray)."""
                c0 = cidx_t[:, (2 * ctab) * 32:(2 * ctab) * 32 + 32]
                c1 = cidx_t[:, (2 * ctab + 1) * 32:(2 * ctab + 1) * 32 + 32]
                G0 = gpool.tile([128, CUT], f32, tag="G0")
                G1 = gpool.tile([128, CUT], f32, tag="G1")
                nc.gpsimd.ap_gather(out_ap=G0[:], in_ap=R01[:], idxs_ap=c0,
                                    channels=128, num_elems=w_al, d=1, num_idxs=CUT)
                nc.gpsimd.ap_gather(out_ap=G1[:], in_ap=R01[:], idxs_ap=c1,
                                    channels=128, num_elems=w_al, d=1, num_idxs=CUT)
                H0 = gpool.tile([64, CUT], f32, tag="H0")
                H1 = gpool.tile([64, CUT], f32, tag="H1")
                nc.gpsimd.ap_gather(out_ap=H0[:], in_ap=R2[:], idxs_ap=c0[:64],
                                    channels=64, num_elems=w_al, d=1, num_idxs=CUT)
                nc.gpsimd.ap_gather(out_ap=H1[:], in_ap=R2[:], idxs_ap=c1[:64],
                                    channels=64, num_elems=w_al, d=1, num_idxs=CUT)
                # O = G0*(1-wx) + G1*wx
                O01 = opool.tile([128, CUT], f32, tag="O01")
                O2 = opool.tile([64, CUT], f32, tag="O2")
                nc.vector.tensor_tensor(out=O01[:], in0=G0[:], in1=wxp[:, 0:CUT], op=MUL)
                nc.vector.tensor_tensor(out=G1[:], in0=G1[:], in1=wxp[:, CUT:], op=MUL)
                nc.vector.tensor_tensor(out=O01[:], in0=O01[:], in1=G1[:], op=ADD)
                nc.vector.tensor_tensor(out=O2[:], in0=H0[:], in1=wxp[:64, 0:CUT], op=MUL)
                nc.vector.tensor_tensor(out=H1[:], in0=H1[:], in1=wxp[:64, CUT:], op=MUL)
                nc.vector.tensor_tensor(out=O2[:], in0=O2[:], in1=H1[:], op=ADD)

                if not gray_only:
                    nc.sync.dma_start(out=out_ap(kout, 0, nch=2), in_=O01[:])
                    nc.sync.dma_start(out=out_ap(kout, 2), in_=O2[:])
                if gray or gray_only:
                    kg = kout + 1 if not gray_only else kout
                    ch1 = opool.tile([64, CUT], f32, tag="ch1")
                    nc.scalar.copy(out=ch1[:], in_=O01[64:128, :])
                    g = opool.tile([64, CUT], f32, tag="gray")
                    nc.scalar.mul(out=g[:], in_=O01[:64, :], mul=float(GRAY_W[0]))
                    nc.vector.scalar_tensor_tensor(out=g[:], in0=ch1[:],
                                                   scalar=float(GRAY_W[1]), in1=g[:],
                                                   op0=MUL, op1=ADD)
                    nc.vector.scalar_tensor_tensor(out=g[:], in0=O2[:],
                                                   scalar=float(GRAY_W[2]), in1=g[:],
                                                   op0=MUL, op1=ADD)
                    for c in range(3):
                        nc.sync.dma_start(out=out_ap(kg, c), in_=g[:])

            # ---- per-resample pipeline (reps>1 only for benchmarking) ----
            for _rep in range(reps):
              gq = 0
              for r, (cx0, w_al) in enumerate(geom):
                  R01 = rpool.tile([128, w_al], f32, tag="R01")
                  R2 = rpool.tile([64, w_al], f32, tag="R2")
                  wyc0 = wyt_t[:, 2 * r:2 * r + 1]
                  wyc1 = wyt_t[:, 2 * r + 1:2 * r + 2]

                  nchunk = (w_al + CHUNK - 1) // CHUNK
                  for ch in range(nchunk):
                      c_lo = ch * CHUNK
                      c_w = min(CHUNK, w_al - c_lo)
                      T = tpool.tile([128, 3, c_w], f32, tag="T")
                      nc.gpsimd.dma_gather(
                          out_ap=T[:],
                          in_ap=img_rows[:, cx0 + c_lo: cx0 + c_lo + c_w],
                          idxs_ap=ridx_t[:, r * 24:r * 24 + 24],
                          num_idxs=384,
                          num_idxs_reg=384,
                          elem_size=c_w,
                          elem_step=W,
                          single_packet=SINGLE_PACKET,
                          queue_num=gq % 4,
                      )
                      gq += 1
                      # row combine chunk
                      nc.scalar.mul(out=R01[:, c_lo:c_lo + c_w],
                                    in_=T[:, 0, :], mul=wyc0)
                      nc.vector.scalar_tensor_tensor(out=R01[:, c_lo:c_lo + c_w],
                                                     in0=T[:, 1, :], scalar=wyc1,
                                                     in1=R01[:, c_lo:c_lo + c_w],
                                                     op0=MUL, op1=ADD)
                      C2b = tpool.tile([64, c_w], f32, tag="C2b")
                      nc.scalar.copy(out=C2b[:], in_=T[64:128, 2, :])
                      nc.scalar.mul(out=R2[:, c_lo:c_lo + c_w],
                                    in_=T[:64, 2, :], mul=wyc0[:64])
                      nc.vector.scalar_tensor_tensor(out=R2[:, c_lo:c_lo + c_w],
                                                     in0=C2b[:], scalar=wyc1[:64],
                                                     in1=R2[:, c_lo:c_lo + c_w],
                                                     op0=MUL, op1=ADD)

                  if r == 0:
                      wxp = wx_bcast(0)
                      col_stage(R01, R2, w_al, ctab=0, wxp=wxp, kout=0, gray=True)
                      wxr = wx_bcast(NSPEC)  # reversed wx pair
                      col_stage(R01, R2, w_al, ctab=NSPEC, wxp=wxr, kout=2, gray=True)
                  else:
                      wxp = wx_bcast(r)
                      kout = 3 + r            # inner k -> out[4 + (r-1)]
                      col_stage(R01, R2, w_al, ctab=r, wxp=wxp, kout=kout,
                                gray_only=(r == 1))
    return nc


# --------------------------------------------------------------------------
# table construction
# --------------------------------------------------------------------------

def _core_tables(params, core):
    r0 = core * STRIP
    ridx_cols = []
    wy_cols = []
    for p in params:
        y0s = p["y0"][r0:r0 + STRIP].astype(np.int32)
        y1s = p["y1"][r0:r0 + STRIP].astype(np.int32)
        idx = np.zeros(384, np.int32)
        for c2 in range(2):
            idx[c2 * 64:c2 * 64 + 64] = c2 * H + y0s
            idx[128 + c2 * 64:128 + c2 * 64 + 64] = c2 * H + y1s
        idx[256:256 + 64] = 2 * H + y0s
        idx[320:320 + 64] = 2 * H + y1s
        ridx_cols.append(_wrap16(idx))
        wys = p["wy"][r0:r0 + STRIP].astype(np.float32)
        one_m = (np.float32(1.0) - wys).astype(np.float32)
        wy_cols.append(np.stack([np.concatenate([one_m, one_m]),
                                 np.concatenate([wys, wys])], axis=1))
    ridx_all = np.concatenate(ridx_cols, axis=1)                    # [128, 13*24]
    wyt = np.concatenate(wy_cols, axis=1).astype(np.float32)        # [128, 26]
    return ridx_all, wyt


def _shared_tables(params):
    cidx_cols = []
    wx_rows = []
    for p in params:
        gx0 = (p["x0"] - p["cx0"]).astype(np.int32)
        gx1 = (p["x1"] - p["cx0"]).astype(np.int32)
        cidx_cols.append(_wrap16(gx0))
        cidx_cols.append(_wrap16(gx1))
        wxs = p["wx"].astype(np.float32)
        wx_rows.append(np.concatenate([(np.float32(1.0) - wxs), wxs]))
    # reversed tables for the overview flip
    p0 = params[0]
    cidx_cols.append(_wrap16((p0["x0"] - p0["cx0"])[::-1].astype(np.int32)))
    cidx_cols.append(_wrap16((p0["x1"] - p0["cx0"])[::-1].astype(np.int32)))
    wxr = p0["wx"][::-1].astype(np.float32)
    wx_rows.append(np.concatenate([(np.float32(1.0) - wxr), wxr]))
    ones_row = np.ones(1024, np.float32)
    wx_rows.append(ones_row)
    cidx_all = np.concatenate(cidx_cols, axis=1)                    # [128, 28*32]
    wxt = np.stack(wx_rows).astype(np.float32)                      # [15, 1024]
    return cidx_all, wxt


# --------------------------------------------------------------------------
# entry point
# --------------------------------------------------------------------------

def _run(img, specs, trace=False):
    from concourse.bass_utils import run_bass_kernel_spmd

    params = _params(specs)
    geom = tuple((p["cx0"], p["w_al"]) for p in params)

    if geom in _CACHE:
        nc = _CACHE[geom]
    else:
        nc = _build_bass(geom)
        nc.compile()
        _CACHE[geom] = nc

    cidx_all, wxt = _shared_tables(params)
    in_maps = []
    for core in range(NCORES):
        ridx_all, wyt = _core_tables(params, core)
        in_maps.append({
            "img": img,
            "ridx": ridx_all,
            "cidx": cidx_all,
            "wyt": wyt,
            "wxt": wxt,
        })

    r = run_bass_kernel_spmd(nc, in_maps, core_ids=list(range(NCORES)),
                             trace=trace)
    strips = [r.results[c]["out"] for c in range(NCORES)]
    out = np.concatenate(strips, axis=2)
    return out, r


def kernel(**inputs):
    img = np.ascontiguousarray(np.asarray(inputs["input"], np.float32)[0])
    sizes = np.asarray(inputs["sizes"])
    offy = np.asarray(inputs["offy"])
    offx = np.asarray(inputs["offx"])
    specs = _specs_from_inputs(sizes, offy, offx)
    out, _ = _run(img, specs, trace=bool(int(os.environ.get("KERNEL_TRACE", "0"))))
    return out.astype(np.float32)



# revision 15
# speedup vs baseline: 487.7960x; 487.7960x over previous
"""DangoCutouts Trainium2 kernel.

Computes reference:
    out[16, 3, 512, 512] =
      [full, gray(full), flip(full), gray(flip(full)), inner_0..11]
    where full = bilinear_resize(img, 4096 -> 512),
          inner_k = bilinear_resize(img[offy_k:+s_k, offx_k:+s_k] -> 512),
          inner_0 additionally grayscaled.

Strategy (8 NeuronCores, data-parallel over output rows):
  Core c computes output rows [64c, 64c+64) of all 16 outputs.
  13 distinct resamples (full + 12 inner). Per resample, per core:
    1. Row gather (dma_gather, SWDGE, 4 queues round-robin): T[128, 3, cw]
       where partition p = (c2, i): c2 in {ch0, ch1}, i = strip row.
       free q-slots: q0 = y0-row, q1 = y1-row of channel c2;
       q2 = ch2 rows (p<64: y0, p>=64: y1).
    2. Row combine: R01 = T0*(1-wy) + T1*wy (Act mul + DVE fused mul-add);
       ch2 via cross-partition copy then same combine -> R2[64, W].
    3. Column stage:
       - full resample (r=0): wy = wx = 0.5 exactly (4096->512 is a 2x2
         box filter at stride 8), so R01 = T0+T1 and the column stage is
         strided DVE adds: O = (R[:,3::8]+R[:,4::8]) * 0.25. No gathers.
         flip(full) / gray(flip) written via reversed-AP DMA stores.
       - inner: single pair-interleaved gpsimd ap_gather per plane
         (idx = x0[0],x1[0],x0[1],x1[1],...) then strided DVE combine
         with wx broadcast to all partitions via a K=1 PE matmul (PSUM).
    4. gray = weighted channel sum on-chip.

All index/weight tables are computed on host in float32 exactly as the
reference does and passed as per-core runtime inputs; the compiled program
structure depends only on the per-resample column windows (cached).
"""
import os
import numpy as np

CUT = 512
H = W = 4096
GRAY_W = (0.2989, 0.587, 0.114)
N_INNER = 12
NSPEC = 13          # full + 12 inner
STRIP = 64          # output rows per core
NCORES = 8
CHUNK = 2048        # row-gather column chunk (elements)
SINGLE_PACKET = True

_CACHE = {}


# --------------------------------------------------------------------------
# host-side parameter math (replicates reference._crop_resize in float32)
# --------------------------------------------------------------------------

def _bilinear_params(offy, offx, size):
    s = np.float32(size)
    t = (np.arange(CUT, dtype=np.float32) + np.float32(0.5)) * s / np.float32(CUT) \
        - np.float32(0.5)
    y = np.clip(np.float32(offy) + t, np.float32(offy), np.float32(offy) + s - np.float32(1.0))
    x = np.clip(np.float32(offx) + t, np.float32(offx), np.float32(offx) + s - np.float32(1.0))
    y0 = np.floor(y).astype(np.int32)
    x0 = np.floor(x).astype(np.int32)
    y1 = np.minimum(y0 + 1, np.int32(offy) + np.int32(size) - 1)
    x1 = np.minimum(x0 + 1, np.int32(offx) + np.int32(size) - 1)
    wy = (y - y0.astype(np.float32)).astype(np.float32)
    wx = (x - x0.astype(np.float32)).astype(np.float32)
    # match XLA gather out-of-bounds clamp / negative wrap for degenerate inputs
    for a in (y0, y1):
        np.copyto(a, np.where(a < 0, a % H, np.minimum(a, H - 1)))
    for a in (x0, x1):
        np.copyto(a, np.where(a < 0, a % W, np.minimum(a, W - 1)))
    return y0, y1, wy, x0, x1, wx


def _col_window(x0, x1):
    cx0 = int(x0[0])
    w = int(x1[-1]) - cx0 + 1
    w_al = min((w + 63) // 64 * 64, W)
    if cx0 + w_al > W:
        cx0 = W - w_al
    return cx0, w_al


def _wrap16(idx):
    """gpsimd idx-table layout: idx[i] -> [16g + i%16, i//16] for all groups g."""
    idx = np.asarray(idx, np.int16)
    n = len(idx)
    assert n % 16 == 0
    cols = n // 16
    tile = np.zeros((128, cols), np.int16)
    blk = idx.reshape(cols, 16).T
    for g in range(8):
        tile[16 * g:16 * g + 16, :] = blk
    return tile


def _specs_from_inputs(sizes, offy, offx):
    specs = [(0, 0, min(H, W))]
    for k in range(N_INNER):
        specs.append((int(offy[k]), int(offx[k]), max(int(sizes[k]), 0)))
    return specs


def _params(specs):
    out = []
    for (oy, ox, s) in specs:
        y0, y1, wy, x0, x1, wx = _bilinear_params(oy, ox, max(s, 1) if s <= 0 else s)
        cx0, w_al = _col_window(x0, x1)
        out.append(dict(y0=y0, y1=y1, wy=wy, x0=x0, x1=x1, wx=wx, cx0=cx0, w_al=w_al))
    return out


# --------------------------------------------------------------------------
# device program
# --------------------------------------------------------------------------

def _build_bass(geom, reps=1, bench=False):
    """geom: tuple of (cx0, w_al) per resample. Returns nc."""
    import concourse.bacc as bacc
    import concourse.mybir as mybir
    from concourse.tile import TileContext

    f32 = mybir.dt.float32
    i16 = mybir.dt.int16
    MUL = mybir.AluOpType.mult
    ADD = mybir.AluOpType.add

    nc = bacc.Bacc("TRN2", target_bir_lowering=False, num_swdge_queues=4)

    img_kind = "Internal" if bench else "ExternalInput"
    img = nc.dram_tensor("img", [3, H, W], f32, kind=img_kind)
    img_rows = img.rearrange("c h w -> (c h) w")
    # per-resample row-gather idx tables, 384 idxs -> [128, 24] each
    ridx = nc.dram_tensor("ridx", [128, NSPEC * 24], i16, kind="ExternalInput")
    # col-gather idx tables: pair-interleaved (x0,x1) per inner resample,
    # 1024 idxs -> [128, 64] each; N_INNER tables
    cidx = nc.dram_tensor("cidx", [128, N_INNER * 64], i16, kind="ExternalInput")
    # row weights: [128, 2*NSPEC]: cols 2k = 1-wy, 2k+1 = wy (dup across c2)
    wyt = nc.dram_tensor("wyt", [128, 2 * NSPEC], f32, kind="ExternalInput")
    # col weights: [N_INNER+1, 1024]: row k: [512 of 1-wx | 512 of wx] for
    # inner k; last row ones (matmul lhsT source)
    wxt = nc.dram_tensor("wxt", [N_INNER + 1, 1024], f32, kind="ExternalInput")

    out_d = nc.dram_tensor("out", [16, 3, STRIP, CUT], f32, kind="ExternalOutput")
    out_rows = out_d.rearrange("k c i j -> (k c i) j")

    def out_ap(k, c, nch=1):
        """[(nch*64), 512] AP at output image k, channel c."""
        base = (k * 3 + c) * STRIP
        return out_rows[base:base + nch * STRIP, :]

    with TileContext(nc) as tc:
        with (
            tc.tile_pool(name="const", bufs=1) as cpool,
            tc.tile_pool(name="tchunk", bufs=2) as tpool,
            tc.tile_pool(name="rslab", bufs=2) as rpool,
            tc.tile_pool(name="gtiles", bufs=2) as gpool,
            tc.tile_pool(name="otiles", bufs=2) as opool,
            tc.tile_pool(name="ovtiles", bufs=1) as ovpool,
            tc.tile_pool(name="psum", bufs=2, space="PSUM") as ppool,
        ):
            # ---- constants ----
            ridx_t = cpool.tile([128, NSPEC * 24], i16)
            nc.sync.dma_start(out=ridx_t[:], in_=ridx[:])
            cidx_t = cpool.tile([128, N_INNER * 64], i16)
            nc.sync.dma_start(out=cidx_t[:], in_=cidx[:])
            wyt_t = cpool.tile([128, 2 * NSPEC], f32)
            nc.sync.dma_start(out=wyt_t[:], in_=wyt[:])
            ones_t = cpool.tile([1, 128], f32)
            nc.sync.dma_start(out=ones_t[:], in_=wxt[N_INNER:N_INNER + 1, 0:128])

            odma_state = [0]

            def odma(out, in_):
                eng = nc.sync if odma_state[0] % 2 == 0 else nc.scalar
                odma_state[0] += 1
                eng.dma_start(out=out, in_=in_)

            def wx_bcast(tab):
                """[128, 1024] PSUM tile: cols 0:512 = 1-wx, 512: = wx."""
                wx_t = gpool.tile([1, 1024], f32, tag="wxrow")
                nc.scalar.dma_start(out=wx_t[:], in_=wxt[tab:tab + 1, :])
                p = ppool.tile([128, 1024], f32, space="PSUM")
                nc.tensor.matmul(out=p[:, 0:CUT], lhsT=ones_t[:], rhs=wx_t[:, 0:CUT],
                                 start=True, stop=True)
                nc.tensor.matmul(out=p[:, CUT:], lhsT=ones_t[:], rhs=wx_t[:, CUT:],
                                 start=True, stop=True)
                return p

            def gray_from(O01, O2, scale=1.0):
                """gray tile [64, CUT] from the channel tiles (pre-scale)."""
                ch1 = opool.tile([64, CUT], f32, tag="ch1")
                nc.scalar.copy(out=ch1[:], in_=O01[64:128, :])
                g = opool.tile([64, CUT], f32, tag="gray")
                nc.scalar.mul(out=g[:], in_=O01[:64, :], mul=float(GRAY_W[0] * scale))
                nc.vector.scalar_tensor_tensor(out=g[:], in0=ch1[:],
                                               scalar=float(GRAY_W[1] * scale),
                                               in1=g[:], op0=MUL, op1=ADD)
                nc.vector.scalar_tensor_tensor(out=g[:], in0=O2[:],
                                               scalar=float(GRAY_W[2] * scale),
                                               in1=g[:], op0=MUL, op1=ADD)
                return g

            def body():
              for r, (cx0, w_al) in enumerate(geom):
                  R01 = rpool.tile([128, w_al], f32, tag="R01")
                  R2 = rpool.tile([64, w_al], f32, tag="R2")
                  wyc0 = wyt_t[:, 2 * r:2 * r + 1]
                  wyc1 = wyt_t[:, 2 * r + 1:2 * r + 2]

                  nchunk = (w_al + CHUNK - 1) // CHUNK
                  for ch in range(nchunk):
                      c_lo = ch * CHUNK
                      c_w = min(CHUNK, w_al - c_lo)
                      T = tpool.tile([128, 3, c_w], f32, tag="T")
                      nc.gpsimd.dma_gather(
                          out_ap=T[:],
                          in_ap=img_rows[:, cx0 + c_lo: cx0 + c_lo + c_w],
                          idxs_ap=ridx_t[:, r * 24:r * 24 + 24],
                          num_idxs=384,
                          num_idxs_reg=384,
                          elem_size=c_w,
                          elem_step=W,
                          single_packet=SINGLE_PACKET,
                          queue_num=(r + ch) % 4,
                      )
                      C2b = tpool.tile([64, c_w], f32, tag="C2b")
                      nc.scalar.copy(out=C2b[:], in_=T[64:128, 2, :])
                      if r == 0:
                          # wy = 0.5 exactly: R = T0 + T1 (x0.25 folded later)
                          nc.vector.tensor_tensor(out=R01[:, c_lo:c_lo + c_w],
                                                  in0=T[:, 0, :], in1=T[:, 1, :],
                                                  op=ADD)
                          nc.vector.tensor_tensor(out=R2[:, c_lo:c_lo + c_w],
                                                  in0=T[:64, 2, :], in1=C2b[:],
                                                  op=ADD)
                      else:
                          nc.scalar.mul(out=R01[:, c_lo:c_lo + c_w],
                                        in_=T[:, 0, :], mul=wyc0)
                          nc.vector.scalar_tensor_tensor(
                              out=R01[:, c_lo:c_lo + c_w],
                              in0=T[:, 1, :], scalar=wyc1,
                              in1=R01[:, c_lo:c_lo + c_w], op0=MUL, op1=ADD)
                          nc.scalar.mul(out=R2[:, c_lo:c_lo + c_w],
                                        in_=T[:64, 2, :], mul=wyc0[:64])
                          nc.vector.scalar_tensor_tensor(
                              out=R2[:, c_lo:c_lo + c_w],
                              in0=C2b[:], scalar=wyc1[:64],
                              in1=R2[:, c_lo:c_lo + c_w], op0=MUL, op1=ADD)

                  if r == 0:
                      # column stage: wx = 0.5 at stride 8 -> strided adds
                      O01 = ovpool.tile([128, CUT], f32, tag="O01")
                      O2 = ovpool.tile([64, CUT], f32, tag="O2")
                      nc.vector.tensor_tensor(out=O01[:], in0=R01[:, 3::8],
                                              in1=R01[:, 4::8], op=ADD)
                      nc.vector.tensor_tensor(out=O2[:], in0=R2[:, 3::8],
                                              in1=R2[:, 4::8], op=ADD)
                      # flipped variants via reversed strided reads (DVE)
                      O01r = ovpool.tile([128, CUT], f32, tag="O01r")
                      O2r = ovpool.tile([64, CUT], f32, tag="O2r")
                      nc.vector.tensor_tensor(out=O01r[:], in0=R01[:, 4091::-8],
                                              in1=R01[:, 4092::-8], op=ADD)
                      nc.vector.tensor_tensor(out=O2r[:], in0=R2[:, 4091::-8],
                                              in1=R2[:, 4092::-8], op=ADD)
                      # gray of the *final* values = gray(Ox0.25)
                      g = gray_from(O01, O2, scale=0.25)
                      gr = ovpool.tile([64, CUT], f32, tag="grayr")
                      nc.vector.tensor_copy(out=gr[:], in_=g[:, ::-1])
                      Of01 = ovpool.tile([128, CUT], f32, tag="Of01")
                      Of2 = ovpool.tile([64, CUT], f32, tag="Of2")
                      Of01r = ovpool.tile([128, CUT], f32, tag="Of01r")
                      Of2r = ovpool.tile([64, CUT], f32, tag="Of2r")
                      nc.scalar.mul(out=Of01[:], in_=O01[:], mul=0.25)
                      nc.scalar.mul(out=Of2[:], in_=O2[:], mul=0.25)
                      nc.scalar.mul(out=Of01r[:], in_=O01r[:], mul=0.25)
                      nc.scalar.mul(out=Of2r[:], in_=O2r[:], mul=0.25)
                      # out0 = full, out1 = gray, out2 = flip, out3 = gray flip
                      odma(out_ap(0, 0, nch=2), Of01[:])
                      odma(out_ap(0, 2), Of2[:])
                      for c in range(3):
                          odma(out_ap(1, c), g[:])
                      odma(out_ap(2, 0, nch=2), Of01r[:])
                      odma(out_ap(2, 2), Of2r[:])
                      for c in range(3):
                          odma(out_ap(3, c), gr[:])
                  else:
                      wxp = wx_bcast(r - 1)
                      ct = cidx_t[:, (r - 1) * 64:(r - 1) * 64 + 64]
                      G = gpool.tile([128, 2 * CUT], f32, tag="G")
                      Hm = gpool.tile([64, 2 * CUT], f32, tag="Hm")
                      nc.gpsimd.ap_gather(out_ap=G[:], in_ap=R01[:], idxs_ap=ct,
                                          channels=128, num_elems=w_al, d=1,
                                          num_idxs=2 * CUT)
                      nc.gpsimd.ap_gather(out_ap=Hm[:], in_ap=R2[:],
                                          idxs_ap=ct[:64], channels=64,
                                          num_elems=w_al, d=1, num_idxs=2 * CUT)
                      # O = G0*(1-wx) + G1*wx  (strided pair reads, wx in PSUM)
                      O01 = opool.tile([128, CUT], f32, tag="iO01")
                      O2 = opool.tile([64, CUT], f32, tag="iO2")
                      t01 = opool.tile([128, CUT], f32, tag="t01")
                      t2 = opool.tile([64, CUT], f32, tag="t2")
                      nc.vector.tensor_tensor(out=O01[:], in0=G[:, 0::2],
                                              in1=wxp[:, 0:CUT], op=MUL)
                      nc.vector.tensor_tensor(out=t01[:], in0=G[:, 1::2],
                                              in1=wxp[:, CUT:], op=MUL)
                      nc.vector.tensor_tensor(out=O01[:], in0=O01[:], in1=t01[:],
                                              op=ADD)
                      nc.vector.tensor_tensor(out=O2[:], in0=Hm[:, 0::2],
                                              in1=wxp[:64, 0:CUT], op=MUL)
                      nc.vector.tensor_tensor(out=t2[:], in0=Hm[:, 1::2],
                                              in1=wxp[:64, CUT:], op=MUL)
                      nc.vector.tensor_tensor(out=O2[:], in0=O2[:], in1=t2[:],
                                              op=ADD)
                      kout = 3 + r            # inner k -> out[4 + (r-1)]
                      if r == 1:
                          g = gray_from(O01, O2)
                          for c in range(3):
                              odma(out_ap(kout, c), g[:])
                      else:
                          odma(out_ap(kout, 0, nch=2), O01[:])
                          odma(out_ap(kout, 2), O2[:])

            if bench:
                with tc.For_i(0, reps) as _i:
                    body()
            else:
                for _rep in range(reps):
                    body()
    return nc


# --------------------------------------------------------------------------
# table construction
# --------------------------------------------------------------------------

def _core_tables(params, core):
    r0 = core * STRIP
    ridx_cols = []
    wy_cols = []
    for p in params:
        y0s = p["y0"][r0:r0 + STRIP].astype(np.int32)
        y1s = p["y1"][r0:r0 + STRIP].astype(np.int32)
        idx = np.zeros(384, np.int32)
        for c2 in range(2):
            idx[c2 * 64:c2 * 64 + 64] = c2 * H + y0s
            idx[128 + c2 * 64:128 + c2 * 64 + 64] = c2 * H + y1s
        idx[256:256 + 64] = 2 * H + y0s
        idx[320:320 + 64] = 2 * H + y1s
        ridx_cols.append(_wrap16(idx))
        wys = p["wy"][r0:r0 + STRIP].astype(np.float32)
        one_m = (np.float32(1.0) - wys).astype(np.float32)
        wy_cols.append(np.stack([np.concatenate([one_m, one_m]),
                                 np.concatenate([wys, wys])], axis=1))
    ridx_all = np.concatenate(ridx_cols, axis=1)                    # [128, 13*24]
    wyt = np.concatenate(wy_cols, axis=1).astype(np.float32)        # [128, 26]
    return ridx_all, wyt


def _shared_tables(params):
    cidx_cols = []
    wx_rows = []
    for p in params[1:]:
        gx0 = (p["x0"] - p["cx0"]).astype(np.int32)
        gx1 = (p["x1"] - p["cx0"]).astype(np.int32)
        pair = np.empty(2 * CUT, np.int32)
        pair[0::2] = gx0
        pair[1::2] = gx1
        cidx_cols.append(_wrap16(pair))
        wxs = p["wx"].astype(np.float32)
        wx_rows.append(np.concatenate([(np.float32(1.0) - wxs), wxs]))
    wx_rows.append(np.ones(1024, np.float32))
    cidx_all = np.concatenate(cidx_cols, axis=1)                    # [128, 12*64]
    wxt = np.stack(wx_rows).astype(np.float32)                      # [13, 1024]
    return cidx_all, wxt


# --------------------------------------------------------------------------
# entry point
# --------------------------------------------------------------------------

def _run(img, specs, trace=False):
    from concourse.bass_utils import run_bass_kernel_spmd

    params = _params(specs)
    geom = tuple((p["cx0"], p["w_al"]) for p in params)

    if geom in _CACHE:
        nc = _CACHE[geom]
    else:
        nc = _build_bass(geom)
        nc.compile()
        _CACHE[geom] = nc

    cidx_all, wxt = _shared_tables(params)
    in_maps = []
    for core in range(NCORES):
        ridx_all, wyt = _core_tables(params, core)
        in_maps.append({
            "img": img,
            "ridx": ridx_all,
            "cidx": cidx_all,
            "wyt": wyt,
            "wxt": wxt,
        })

    r = run_bass_kernel_spmd(nc, in_maps, core_ids=list(range(NCORES)),
                             trace=trace)
    strips = [r.results[c]["out"] for c in range(NCORES)]
    out = np.concatenate(strips, axis=2)
    return out, r


def kernel(**inputs):
    img = np.ascontiguousarray(np.asarray(inputs["input"], np.float32)[0])
    sizes = np.asarray(inputs["sizes"])
    offy = np.asarray(inputs["offy"])
    offx = np.asarray(inputs["offx"])
    specs = _specs_from_inputs(sizes, offy, offx)
    out, _ = _run(img, specs, trace=bool(int(os.environ.get("KERNEL_TRACE", "0"))))
    return out.astype(np.float32)


# revision 23
# speedup vs baseline: 1531.9012x; 3.1405x over previous
"""DangoCutouts Trainium2 kernel.

Computes reference:
    out[16, 3, 512, 512] =
      [full, gray(full), flip(full), gray(flip(full)), inner_0..11]
    where full = bilinear_resize(img, 4096 -> 512),
          inner_k = bilinear_resize(img[offy_k:+s_k, offx_k:+s_k] -> 512),
          inner_0 additionally grayscaled.

Strategy (8 NeuronCores, data-parallel over output rows):
  Core c computes output rows [64c, 64c+64) of all 16 outputs.
  13 distinct resamples (full + 12 inner). Per resample, per core:
    1. Row gather (dma_gather, SWDGE, 4 queues round-robin): T[128, 3, cw]
       where partition p = (c2, i): c2 in {ch0, ch1}, i = strip row.
       free q-slots: q0 = y0-row, q1 = y1-row of channel c2;
       q2 = ch2 rows (p<64: y0, p>=64: y1).
    2. Row combine: R01 = T0*(1-wy) + T1*wy (Act mul + DVE fused mul-add)
       written in bf16; ch2 via cross-partition copy then same -> R2[64, W].
    3. Column stage:
       - full resample (r=0): wy = wx = 0.5 exactly (4096->512 is a 2x2
         box filter at stride 8): strided DVE adds, flip via reversed
         strided reads. No gathers.
       - inner: column bilinear = block-sparse matmul. R is XBAR
         DMA-transposed (bf16) into RT[128k, kb, m]; per 128-col k-block a
         host-shipped bf16 weight block Wx[k, j] (<=2 nonzeros per column)
         is matmul'd on the PE, accumulating into PSUM O[rows, j].
         Columns whose two taps straddle a k-block boundary get a
         start=False accumulate matmul from the second block.
    4. gray = weighted channel sum on-chip.

The PE-matmul x-stage replaces gpsimd ap_gather (measured ~29ns/idx on HW,
~710us/core for the gathers) with ~100us of PE time.

All index/weight tables are computed on host in float32 exactly as the
reference does and passed as per-core runtime inputs; the compiled program
structure depends only on the geometry skeleton (cached).
"""
import os
import numpy as np

CUT = 512
H = W = 4096
GRAY_W = (0.2989, 0.587, 0.114)
N_INNER = 12
NSPEC = 13          # full + 12 inner
STRIP = 64          # output rows per core
NCORES = 8
CHUNK = 2048        # row-gather column chunk (elements)
SINGLE_PACKET = True

_CACHE = {}


# --------------------------------------------------------------------------
# host-side parameter math (replicates reference._crop_resize in float32)
# --------------------------------------------------------------------------

def _bilinear_params(offy, offx, size):
    s = np.float32(size)
    t = (np.arange(CUT, dtype=np.float32) + np.float32(0.5)) * s / np.float32(CUT) \
        - np.float32(0.5)
    y = np.clip(np.float32(offy) + t, np.float32(offy), np.float32(offy) + s - np.float32(1.0))
    x = np.clip(np.float32(offx) + t, np.float32(offx), np.float32(offx) + s - np.float32(1.0))
    y0 = np.floor(y).astype(np.int32)
    x0 = np.floor(x).astype(np.int32)
    y1 = np.minimum(y0 + 1, np.int32(offy) + np.int32(size) - 1)
    x1 = np.minimum(x0 + 1, np.int32(offx) + np.int32(size) - 1)
    wy = (y - y0.astype(np.float32)).astype(np.float32)
    wx = (x - x0.astype(np.float32)).astype(np.float32)
    # match XLA gather out-of-bounds clamp / negative wrap for degenerate inputs
    for a in (y0, y1):
        np.copyto(a, np.where(a < 0, a % H, np.minimum(a, H - 1)))
    for a in (x0, x1):
        np.copyto(a, np.where(a < 0, a % W, np.minimum(a, W - 1)))
    return y0, y1, wy, x0, x1, wx


def _col_window(x0, x1):
    cx0 = int(x0[0])
    w = int(x1[-1]) - cx0 + 1
    w_al = min((w + 127) // 128 * 128, W)
    if cx0 + w_al > W:
        cx0 = W - w_al
    return cx0, w_al


def _wrap16(idx):
    """gpsimd idx-table layout: idx[i] -> [16g + i%16, i//16] for all groups g."""
    idx = np.asarray(idx, np.int16)
    n = len(idx)
    assert n % 16 == 0
    cols = n // 16
    tile = np.zeros((128, cols), np.int16)
    blk = idx.reshape(cols, 16).T
    for g in range(8):
        tile[16 * g:16 * g + 16, :] = blk
    return tile


def _specs_from_inputs(sizes, offy, offx):
    specs = [(0, 0, min(H, W))]
    for k in range(N_INNER):
        specs.append((int(offy[k]), int(offx[k]), max(int(sizes[k]), 0)))
    return specs


def _params(specs):
    out = []
    for (oy, ox, s) in specs:
        y0, y1, wy, x0, x1, wx = _bilinear_params(oy, ox, max(s, 1) if s <= 0 else s)
        cx0, w_al = _col_window(x0, x1)
        out.append(dict(y0=y0, y1=y1, wy=wy, x0=x0, x1=x1, wx=wx, cx0=cx0, w_al=w_al))
    return out


def _xblocks(p):
    """Per k-block matmul plan + weight columns for one inner resample.

    Returns (blocks, cols): blocks = list of
      (kb, n_acc, jacc_lo, n_main, jmain_lo)  (column offset implicit by
      accumulation order), cols = [n_cols][128] float32 weight columns.
    """
    gx0 = (p["x0"] - p["cx0"]).astype(np.int64)
    gx1 = (p["x1"] - p["cx0"]).astype(np.int64)
    wx = p["wx"].astype(np.float32)
    b0 = gx0 // 128
    b1 = gx1 // 128
    nblk = p["w_al"] // 128
    blocks = []
    cols = []
    for kb in range(nblk):
        acc_j = np.nonzero((b0 < kb) & (b1 == kb))[0]
        main_j = np.nonzero(b0 == kb)[0]
        n_acc, n_main = len(acc_j), len(main_j)
        if n_acc == 0 and n_main == 0:
            continue
        if n_acc:
            assert acc_j[-1] - acc_j[0] + 1 == n_acc   # contiguous
        if n_main:
            assert main_j[-1] - main_j[0] + 1 == n_main
        if n_acc and n_main:
            assert acc_j[-1] + 1 == main_j[0]
        for j in acc_j:
            c = np.zeros(128, np.float32)
            c[gx1[j] - 128 * kb] += wx[j]
            cols.append(c)
        for j in main_j:
            c = np.zeros(128, np.float32)
            c[gx0[j] - 128 * kb] += np.float32(1.0) - wx[j]
            if b1[j] == kb:
                c[gx1[j] - 128 * kb] += wx[j]
            cols.append(c)
        blocks.append((int(kb), int(n_acc), int(acc_j[0]) if n_acc else 0,
                       int(n_main), int(main_j[0]) if n_main else 0))
    return blocks, cols


def _plan(params):
    """Compile-relevant skeleton + runtime weight tensor."""
    skeleton = []
    all_cols = []
    for p in params[1:]:
        blocks, cols = _xblocks(p)
        coff = len(all_cols)
        all_cols.extend(cols)
        skeleton.append((p["cx0"], p["w_al"], int(coff), tuple(blocks)))
    ncols_pad = (len(all_cols) + 127) // 128 * 128
    wxb = np.zeros((128, ncols_pad), np.float32)
    for i, c in enumerate(all_cols):
        wxb[:, i] = c
    skel = ((params[0]["cx0"], params[0]["w_al"]), tuple(skeleton), ncols_pad)
    return skel, wxb


# --------------------------------------------------------------------------
# device program
# --------------------------------------------------------------------------

def _build_bass(skel, reps=1, bench=False):
    import concourse.bacc as bacc
    import concourse.mybir as mybir
    from concourse.tile import TileContext

    f32 = mybir.dt.float32
    bf16 = mybir.dt.bfloat16
    i16 = mybir.dt.int16
    MUL = mybir.AluOpType.mult
    ADD = mybir.AluOpType.add

    (cx0_full, wal_full), inner_skel, ncols = skel

    nc = bacc.Bacc("TRN2", target_bir_lowering=False, num_swdge_queues=4)

    img_kind = "Internal" if bench else "ExternalInput"
    img = nc.dram_tensor("img", [3, H, W], f32, kind=img_kind)
    img_rows = img.rearrange("c h w -> (c h) w")
    ridx = nc.dram_tensor("ridx", [128, NSPEC * 24], i16, kind="ExternalInput")
    wyt = nc.dram_tensor("wyt", [128, 2 * NSPEC], f32, kind="ExternalInput")
    wxb_d = nc.dram_tensor("wxb", [128, ncols], f32, kind="ExternalInput")

    out_d = nc.dram_tensor("out", [16, 3, STRIP, CUT], f32, kind="ExternalOutput")
    out_rows = out_d.rearrange("k c i j -> (k c i) j")

    def out_ap(k, c, nch=1):
        base = (k * 3 + c) * STRIP
        return out_rows[base:base + nch * STRIP, :]

    with TileContext(nc) as tc:
        with (
            tc.tile_pool(name="const", bufs=1) as cpool,
            tc.tile_pool(name="tchunk", bufs=2) as tpool,
            tc.tile_pool(name="rslab", bufs=2) as rpool,
            tc.tile_pool(name="rtslab", bufs=2) as rtpool,
            tc.tile_pool(name="otiles", bufs=2) as opool,
            tc.tile_pool(name="ovtiles", bufs=1) as ovpool,
            tc.tile_pool(name="psum", bufs=2, space="PSUM") as ppool,
        ):
            # ---- constants ----
            ridx_t = cpool.tile([128, NSPEC * 24], i16)
            nc.sync.dma_start(out=ridx_t[:], in_=ridx[:])
            wyt_t = cpool.tile([128, 2 * NSPEC], f32)
            nc.sync.dma_start(out=wyt_t[:], in_=wyt[:])
            # casting DMA (SWDGE): f32 DRAM -> bf16 SBUF
            wxb_t = cpool.tile([128, ncols], bf16)
            nc.gpsimd.dma_start(out=wxb_t[:], in_=wxb_d[:])

            odma_state = [0]

            def odma(out, in_):
                eng = nc.sync if odma_state[0] % 2 == 0 else nc.scalar
                odma_state[0] += 1
                eng.dma_start(out=out, in_=in_)

            def gray_from(O01, O2, scale=1.0):
                """gray tile [64, CUT] from the channel tiles (pre-scale)."""
                ch1 = opool.tile([64, CUT], f32, tag="ch1")
                nc.scalar.copy(out=ch1[:], in_=O01[64:128, :])
                g = opool.tile([64, CUT], f32, tag="gray")
                nc.scalar.mul(out=g[:], in_=O01[:64, :], mul=float(GRAY_W[0] * scale))
                nc.vector.scalar_tensor_tensor(out=g[:], in0=ch1[:],
                                               scalar=float(GRAY_W[1] * scale),
                                               in1=g[:], op0=MUL, op1=ADD)
                nc.vector.scalar_tensor_tensor(out=g[:], in0=O2[:],
                                               scalar=float(GRAY_W[2] * scale),
                                               in1=g[:], op0=MUL, op1=ADD)
                return g

            def fetch_and_ycombine(r, cx0, w_al, out_dt):
                """Row gather + y-combine -> (R01[128,w_al], R2[64,w_al])."""
                R01 = rpool.tile([128, w_al], out_dt, tag="R01")
                R2 = rpool.tile([64, w_al], out_dt, tag="R2")
                wyc0 = wyt_t[:, 2 * r:2 * r + 1]
                wyc1 = wyt_t[:, 2 * r + 1:2 * r + 2]
                nchunk = (w_al + CHUNK - 1) // CHUNK
                for ch in range(nchunk):
                    c_lo = ch * CHUNK
                    c_w = min(CHUNK, w_al - c_lo)
                    T = tpool.tile([128, 3, c_w], f32, tag="T")
                    nc.gpsimd.dma_gather(
                        out_ap=T[:],
                        in_ap=img_rows[:, cx0 + c_lo: cx0 + c_lo + c_w],
                        idxs_ap=ridx_t[:, r * 24:r * 24 + 24],
                        num_idxs=384,
                        num_idxs_reg=384,
                        elem_size=c_w,
                        elem_step=W,
                        single_packet=SINGLE_PACKET,
                        queue_num=(r + ch) % 4,
                    )
                    C2b = tpool.tile([64, c_w], f32, tag="C2b")
                    nc.scalar.copy(out=C2b[:], in_=T[64:128, 2, :])
                    if r == 0:
                        # wy = 0.5 exactly: R = T0 + T1 (x0.25 folded later)
                        nc.vector.tensor_tensor(out=R01[:, c_lo:c_lo + c_w],
                                                in0=T[:, 0, :], in1=T[:, 1, :],
                                                op=ADD)
                        nc.vector.tensor_tensor(out=R2[:, c_lo:c_lo + c_w],
                                                in0=T[:64, 2, :], in1=C2b[:],
                                                op=ADD)
                    else:
                        nc.scalar.mul(out=R01[:, c_lo:c_lo + c_w],
                                      in_=T[:, 0, :], mul=wyc0)
                        nc.vector.scalar_tensor_tensor(
                            out=R01[:, c_lo:c_lo + c_w],
                            in0=T[:, 1, :], scalar=wyc1,
                            in1=R01[:, c_lo:c_lo + c_w], op0=MUL, op1=ADD)
                        nc.scalar.mul(out=R2[:, c_lo:c_lo + c_w],
                                      in_=T[:64, 2, :], mul=wyc0[:64])
                        nc.vector.scalar_tensor_tensor(
                            out=R2[:, c_lo:c_lo + c_w],
                            in0=C2b[:], scalar=wyc1[:64],
                            in1=R2[:, c_lo:c_lo + c_w], op0=MUL, op1=ADD)
                return R01, R2

            def body():
                # ---------------- overview (r=0) ----------------
                cx0, w_al = cx0_full, wal_full
                R01, R2 = fetch_and_ycombine(0, cx0, w_al, f32)
                O01 = ovpool.tile([128, CUT], f32, tag="O01")
                O2 = ovpool.tile([64, CUT], f32, tag="O2")
                nc.vector.tensor_tensor(out=O01[:], in0=R01[:, 3::8],
                                        in1=R01[:, 4::8], op=ADD)
                nc.vector.tensor_tensor(out=O2[:], in0=R2[:, 3::8],
                                        in1=R2[:, 4::8], op=ADD)
                O01r = ovpool.tile([128, CUT], f32, tag="O01r")
                O2r = ovpool.tile([64, CUT], f32, tag="O2r")
                nc.vector.tensor_tensor(out=O01r[:], in0=R01[:, 4091::-8],
                                        in1=R01[:, 4092::-8], op=ADD)
                nc.vector.tensor_tensor(out=O2r[:], in0=R2[:, 4091::-8],
                                        in1=R2[:, 4092::-8], op=ADD)
                g = gray_from(O01, O2, scale=0.25)
                gr = ovpool.tile([64, CUT], f32, tag="grayr")
                nc.vector.tensor_copy(out=gr[:], in_=g[:, ::-1])
                Of01 = ovpool.tile([128, CUT], f32, tag="Of01")
                Of2 = ovpool.tile([64, CUT], f32, tag="Of2")
                Of01r = ovpool.tile([128, CUT], f32, tag="Of01r")
                Of2r = ovpool.tile([64, CUT], f32, tag="Of2r")
                nc.scalar.mul(out=Of01[:], in_=O01[:], mul=0.25)
                nc.scalar.mul(out=Of2[:], in_=O2[:], mul=0.25)
                nc.scalar.mul(out=Of01r[:], in_=O01r[:], mul=0.25)
                nc.scalar.mul(out=Of2r[:], in_=O2r[:], mul=0.25)
                odma(out_ap(0, 0, nch=2), Of01[:])
                odma(out_ap(0, 2), Of2[:])
                for c in range(3):
                    odma(out_ap(1, c), g[:])
                odma(out_ap(2, 0, nch=2), Of01r[:])
                odma(out_ap(2, 2), Of2r[:])
                for c in range(3):
                    odma(out_ap(3, c), gr[:])

                # ---------------- inner (r=1..12) ----------------
                for ri, (cx0, w_al, coff, blocks) in enumerate(inner_skel):
                    r = ri + 1
                    R01, R2 = fetch_and_ycombine(r, cx0, w_al, bf16)
                    nblk = w_al // 128
                    RT01 = rtpool.tile([128, nblk, 128], bf16, tag="RT01")
                    RT2 = rtpool.tile([128, nblk, 64], bf16, tag="RT2")
                    nc.scalar.dma_start_transpose(RT01[:], R01[:])
                    nc.scalar.dma_start_transpose(RT2[:], R2[:])
                    O01p = ppool.tile([128, CUT], f32, space="PSUM")
                    O2p = ppool.tile([64, CUT], f32, space="PSUM")
                    c = coff
                    for (kb, n_acc, jacc_lo, n_main, jmain_lo) in blocks:
                        if n_acc:
                            rhs = wxb_t[:, c:c + n_acc]
                            nc.tensor.matmul(
                                out=O01p[:, jacc_lo:jacc_lo + n_acc],
                                lhsT=RT01[:, kb, :], rhs=rhs,
                                start=False, stop=True, skip_group_check=True)
                            nc.tensor.matmul(
                                out=O2p[:, jacc_lo:jacc_lo + n_acc],
                                lhsT=RT2[:, kb, :], rhs=rhs,
                                start=False, stop=True, skip_group_check=True)
                            c += n_acc
                        if n_main:
                            rhs = wxb_t[:, c:c + n_main]
                            nc.tensor.matmul(
                                out=O01p[:, jmain_lo:jmain_lo + n_main],
                                lhsT=RT01[:, kb, :], rhs=rhs,
                                start=True, stop=True, skip_group_check=True)
                            nc.tensor.matmul(
                                out=O2p[:, jmain_lo:jmain_lo + n_main],
                                lhsT=RT2[:, kb, :], rhs=rhs,
                                start=True, stop=True, skip_group_check=True)
                            c += n_main
                    O01 = opool.tile([128, CUT], f32, tag="iO01")
                    O2 = opool.tile([64, CUT], f32, tag="iO2")
                    nc.scalar.copy(out=O01[:], in_=O01p[:])
                    nc.vector.tensor_copy(out=O2[:], in_=O2p[:])
                    kout = 3 + r            # inner k -> out[4 + (r-1)]
                    if r == 1:
                        g = gray_from(O01, O2)
                        for cch in range(3):
                            odma(out_ap(kout, cch), g[:])
                    else:
                        odma(out_ap(kout, 0, nch=2), O01[:])
                        odma(out_ap(kout, 2), O2[:])

            if bench:
                with tc.For_i(0, reps) as _i:
                    body()
            else:
                for _rep in range(reps):
                    body()
    return nc


# --------------------------------------------------------------------------
# table construction
# --------------------------------------------------------------------------

def _core_tables(params, core):
    r0 = core * STRIP
    ridx_cols = []
    wy_cols = []
    for p in params:
        y0s = p["y0"][r0:r0 + STRIP].astype(np.int32)
        y1s = p["y1"][r0:r0 + STRIP].astype(np.int32)
        idx = np.zeros(384, np.int32)
        for c2 in range(2):
            idx[c2 * 64:c2 * 64 + 64] = c2 * H + y0s
            idx[128 + c2 * 64:128 + c2 * 64 + 64] = c2 * H + y1s
        idx[256:256 + 64] = 2 * H + y0s
        idx[320:320 + 64] = 2 * H + y1s
        ridx_cols.append(_wrap16(idx))
        wys = p["wy"][r0:r0 + STRIP].astype(np.float32)
        one_m = (np.float32(1.0) - wys).astype(np.float32)
        wy_cols.append(np.stack([np.concatenate([one_m, one_m]),
                                 np.concatenate([wys, wys])], axis=1))
    ridx_all = np.concatenate(ridx_cols, axis=1)                    # [128, 13*24]
    wyt = np.concatenate(wy_cols, axis=1).astype(np.float32)        # [128, 26]
    return ridx_all, wyt


# --------------------------------------------------------------------------
# entry point
# --------------------------------------------------------------------------

def _run(img, specs, trace=False):
    from concourse.bass_utils import run_bass_kernel_spmd

    params = _params(specs)
    skel, wxb = _plan(params)

    if skel in _CACHE:
        nc = _CACHE[skel]
    else:
        nc = _build_bass(skel)
        nc.compile()
        _CACHE[skel] = nc

    in_maps = []
    for core in range(NCORES):
        ridx_all, wyt = _core_tables(params, core)
        in_maps.append({
            "img": img,
            "ridx": ridx_all,
            "wyt": wyt,
            "wxb": wxb,
        })

    r = run_bass_kernel_spmd(nc, in_maps, core_ids=list(range(NCORES)),
                             trace=trace)
    strips = [r.results[c]["out"] for c in range(NCORES)]
    out = np.concatenate(strips, axis=2)
    return out, r


def kernel(**inputs):
    img = np.ascontiguousarray(np.asarray(inputs["input"], np.float32)[0])
    sizes = np.asarray(inputs["sizes"])
    offy = np.asarray(inputs["offy"])
    offx = np.asarray(inputs["offx"])
    specs = _specs_from_inputs(sizes, offy, offx)
    out, _ = _run(img, specs, trace=bool(int(os.environ.get("KERNEL_TRACE", "0"))))
    return out.astype(np.float32)


# revision 33
# speedup vs baseline: 2342.9657x; 1.5294x over previous
"""DangoCutouts Trainium2 kernel.

Computes reference:
    out[16, 3, 512, 512] =
      [full, gray(full), flip(full), gray(flip(full)), inner_0..11]
    where full = bilinear_resize(img, 4096 -> 512),
          inner_k = bilinear_resize(img[offy_k:+s_k, offx_k:+s_k] -> 512),
          inner_0 additionally grayscaled.

Strategy (8 NeuronCores, data-parallel over output rows):
  Core c computes output rows [64c, 64c+64) of all 16 outputs.
  13 distinct resamples (full + 12 inner). Per resample, per core:
    1. Row gather (dma_gather, SWDGE, 4 queues round-robin): T[128, 3, cw]
       where partition p = (c2, i): c2 in {ch0, ch1}, i = strip row.
       free q-slots: q0 = y0-row, q1 = y1-row of channel c2;
       q2 = ch2 rows (p<64: y0, p>=64: y1).
    2. Row combine: R01 = T0*(1-wy) + T1*wy (Act mul + DVE fused mul-add)
       written in bf16; ch2 via cross-partition copy then same -> R2[64, W].
    3. Column stage:
       - full resample (r=0): wy = wx = 0.5 exactly (4096->512 is a 2x2
         box filter at stride 8): strided DVE adds, flip via reversed
         strided reads. No gathers.
       - inner: column bilinear = block-sparse matmul. R is XBAR
         DMA-transposed (bf16) into RT[128k, kb, m]; per 128-col k-block a
         host-shipped bf16 weight block Wx[k, j] (<=2 nonzeros per column)
         is matmul'd on the PE, accumulating into PSUM O[rows, j].
         Columns whose two taps straddle a k-block boundary get a
         start=False accumulate matmul from the second block.
    4. gray = weighted channel sum on-chip.

The PE-matmul x-stage replaces gpsimd ap_gather (measured ~29ns/idx on HW,
~710us/core for the gathers) with ~100us of PE time.

All index/weight tables are computed on host in float32 exactly as the
reference does and passed as per-core runtime inputs; the compiled program
structure depends only on the geometry skeleton (cached).
"""
import os
import numpy as np

CUT = 512
H = W = 4096
GRAY_W = (0.2989, 0.587, 0.114)
N_INNER = 12
NSPEC = 13          # full + 12 inner
STRIP = 64          # output rows per core
NCORES = 8
CHUNK = 2048        # row-gather column chunk (elements)
SINGLE_PACKET = True

_CACHE = {}


# --------------------------------------------------------------------------
# host-side parameter math (replicates reference._crop_resize in float32)
# --------------------------------------------------------------------------

def _bilinear_params(offy, offx, size):
    s = np.float32(size)
    t = (np.arange(CUT, dtype=np.float32) + np.float32(0.5)) * s / np.float32(CUT) \
        - np.float32(0.5)
    y = np.clip(np.float32(offy) + t, np.float32(offy), np.float32(offy) + s - np.float32(1.0))
    x = np.clip(np.float32(offx) + t, np.float32(offx), np.float32(offx) + s - np.float32(1.0))
    y0 = np.floor(y).astype(np.int32)
    x0 = np.floor(x).astype(np.int32)
    y1 = np.minimum(y0 + 1, np.int32(offy) + np.int32(size) - 1)
    x1 = np.minimum(x0 + 1, np.int32(offx) + np.int32(size) - 1)
    wy = (y - y0.astype(np.float32)).astype(np.float32)
    wx = (x - x0.astype(np.float32)).astype(np.float32)
    # match XLA gather out-of-bounds clamp / negative wrap for degenerate inputs
    for a in (y0, y1):
        np.copyto(a, np.where(a < 0, a % H, np.minimum(a, H - 1)))
    for a in (x0, x1):
        np.copyto(a, np.where(a < 0, a % W, np.minimum(a, W - 1)))
    return y0, y1, wy, x0, x1, wx


def _col_window(x0, x1):
    cx0 = int(x0[0])
    w = int(x1[-1]) - cx0 + 1
    w_al = min((w + 127) // 128 * 128, W)
    if cx0 + w_al > W:
        cx0 = W - w_al
    return cx0, w_al


def _wrap16(idx):
    """gpsimd idx-table layout: idx[i] -> [16g + i%16, i//16] for all groups g."""
    idx = np.asarray(idx, np.int16)
    n = len(idx)
    assert n % 16 == 0
    cols = n // 16
    tile = np.zeros((128, cols), np.int16)
    blk = idx.reshape(cols, 16).T
    for g in range(8):
        tile[16 * g:16 * g + 16, :] = blk
    return tile


def _specs_from_inputs(sizes, offy, offx):
    specs = [(0, 0, min(H, W))]
    for k in range(N_INNER):
        specs.append((int(offy[k]), int(offx[k]), max(int(sizes[k]), 0)))
    return specs


def _params(specs):
    out = []
    for (oy, ox, s) in specs:
        y0, y1, wy, x0, x1, wx = _bilinear_params(oy, ox, max(s, 1) if s <= 0 else s)
        cx0, w_al = _col_window(x0, x1)
        out.append(dict(y0=y0, y1=y1, wy=wy, x0=x0, x1=x1, wx=wx, cx0=cx0, w_al=w_al))
    return out


def _xblocks(p):
    """Per k-block matmul plan + weight columns for one inner resample.

    Returns (blocks, cols): blocks = list of
      (kb, n_acc, jacc_lo, n_main, jmain_lo)  (column offset implicit by
      accumulation order), cols = [n_cols][128] float32 weight columns.
    """
    gx0 = (p["x0"] - p["cx0"]).astype(np.int64)
    gx1 = (p["x1"] - p["cx0"]).astype(np.int64)
    wx = p["wx"].astype(np.float32)
    b0 = gx0 // 128
    b1 = gx1 // 128
    nblk = p["w_al"] // 128
    blocks = []
    cols = []
    for kb in range(nblk):
        acc_j = np.nonzero((b0 < kb) & (b1 == kb))[0]
        main_j = np.nonzero(b0 == kb)[0]
        n_acc, n_main = len(acc_j), len(main_j)
        if n_acc == 0 and n_main == 0:
            continue
        if n_acc:
            assert acc_j[-1] - acc_j[0] + 1 == n_acc   # contiguous
        if n_main:
            assert main_j[-1] - main_j[0] + 1 == n_main
        if n_acc and n_main:
            assert acc_j[-1] + 1 == main_j[0]
        for j in acc_j:
            c = np.zeros(128, np.float32)
            c[gx1[j] - 128 * kb] += wx[j]
            cols.append(c)
        for j in main_j:
            c = np.zeros(128, np.float32)
            c[gx0[j] - 128 * kb] += np.float32(1.0) - wx[j]
            if b1[j] == kb:
                c[gx1[j] - 128 * kb] += wx[j]
            cols.append(c)
        blocks.append((int(kb), int(n_acc), int(acc_j[0]) if n_acc else 0,
                       int(n_main), int(main_j[0]) if n_main else 0))
    return blocks, cols


def _plan(params):
    """Compile-relevant skeleton + runtime weight tensor."""
    skeleton = []
    all_cols = []
    for p in params[1:]:
        blocks, cols = _xblocks(p)
        coff = len(all_cols)
        all_cols.extend(cols)
        skeleton.append((p["cx0"], p["w_al"], int(coff), tuple(blocks)))
    ncols_pad = (len(all_cols) + 127) // 128 * 128
    wxb = np.zeros((128, ncols_pad), np.float32)
    for i, c in enumerate(all_cols):
        wxb[:, i] = c
    skel = ((params[0]["cx0"], params[0]["w_al"]), tuple(skeleton), ncols_pad)
    return skel, wxb


# --------------------------------------------------------------------------
# device program
# --------------------------------------------------------------------------

def _build_bass(skel, reps=1, bench=False):
    import concourse.bacc as bacc
    import concourse.mybir as mybir
    from concourse.tile import TileContext

    f32 = mybir.dt.float32
    bf16 = mybir.dt.bfloat16
    i16 = mybir.dt.int16
    MUL = mybir.AluOpType.mult
    ADD = mybir.AluOpType.add

    (cx0_full, wal_full), inner_skel, ncols = skel

    nc = bacc.Bacc("TRN2", target_bir_lowering=False, num_swdge_queues=4)

    img_kind = "Internal" if bench else "ExternalInput"
    img = nc.dram_tensor("img", [3, H, W], f32, kind=img_kind)
    img_rows = img.rearrange("c h w -> (c h) w")
    ridx = nc.dram_tensor("ridx", [128, NSPEC * 24], i16, kind="ExternalInput")
    wyt = nc.dram_tensor("wyt", [128, 2 * NSPEC], f32, kind="ExternalInput")
    wxb_d = nc.dram_tensor("wxb", [128, ncols], f32, kind="ExternalInput")

    out_d = nc.dram_tensor("out", [16, 3, STRIP, CUT], f32, kind="ExternalOutput")
    out_rows = out_d.rearrange("k c i j -> (k c i) j")

    def out_ap(k, c, nch=1):
        base = (k * 3 + c) * STRIP
        return out_rows[base:base + nch * STRIP, :]

    with TileContext(nc) as tc:
        with (
            tc.tile_pool(name="const", bufs=1) as cpool,
            tc.tile_pool(name="tchunk", bufs=3) as tpool,
            tc.tile_pool(name="c2chunk", bufs=2) as c2pool,
            tc.tile_pool(name="rslab", bufs=2) as rpool,
            tc.tile_pool(name="rtslab", bufs=2) as rtpool,
            tc.tile_pool(name="otiles", bufs=2) as opool,
            tc.tile_pool(name="ovtiles", bufs=1) as ovpool,
            tc.tile_pool(name="psum", bufs=2, space="PSUM") as ppool,
        ):
            # ---- constants ----
            ridx_t = cpool.tile([128, NSPEC * 24], i16)
            nc.sync.dma_start(out=ridx_t[:], in_=ridx[:])
            wyt_t = cpool.tile([128, 2 * NSPEC], f32)
            nc.sync.dma_start(out=wyt_t[:], in_=wyt[:])
            # casting DMA (SWDGE): f32 DRAM -> bf16 SBUF
            wxb_t = cpool.tile([128, ncols], bf16)
            nc.gpsimd.dma_start(out=wxb_t[:], in_=wxb_d[:])

            odma_state = [0]

            def odma(out, in_):
                eng = nc.sync if odma_state[0] % 2 == 0 else nc.scalar
                odma_state[0] += 1
                eng.dma_start(out=out, in_=in_)

            def gray_from(O01, O2, scale=1.0):
                """gray tile [64, CUT] from the channel tiles (pre-scale)."""
                ch1 = ovpool.tile([64, CUT], f32, tag="ch1")
                nc.scalar.copy(out=ch1[:], in_=O01[64:128, :])
                g = ovpool.tile([64, CUT], f32, tag="gray")
                nc.scalar.mul(out=g[:], in_=O01[:64, :], mul=float(GRAY_W[0] * scale))
                nc.vector.scalar_tensor_tensor(out=g[:], in0=ch1[:],
                                               scalar=float(GRAY_W[1] * scale),
                                               in1=g[:], op0=MUL, op1=ADD)
                nc.vector.scalar_tensor_tensor(out=g[:], in0=O2[:],
                                               scalar=float(GRAY_W[2] * scale),
                                               in1=g[:], op0=MUL, op1=ADD)
                return g, ch1

            def fetch_and_ycombine(r, cx0, w_al, out_dt):
                """Row gather + y-combine -> (R01[128,w_al], R2[64,w_al])."""
                R01 = rpool.tile([128, w_al], out_dt, tag="R01")
                R2 = rpool.tile([64, w_al], out_dt, tag="R2")
                wyc0 = wyt_t[:, 2 * r:2 * r + 1]
                wyc1 = wyt_t[:, 2 * r + 1:2 * r + 2]
                nchunk = (w_al + CHUNK - 1) // CHUNK
                for ch in range(nchunk):
                    c_lo = ch * CHUNK
                    c_w = min(CHUNK, w_al - c_lo)
                    T = tpool.tile([128, 3, c_w], f32, tag="T")
                    nc.gpsimd.dma_gather(
                        out_ap=T[:],
                        in_ap=img_rows[:, cx0 + c_lo: cx0 + c_lo + c_w],
                        idxs_ap=ridx_t[:, r * 24:r * 24 + 24],
                        num_idxs=384,
                        num_idxs_reg=384,
                        elem_size=c_w,
                        elem_step=W,
                        single_packet=SINGLE_PACKET,
                        queue_num=(r + ch) % 4,
                    )
                    C2b = c2pool.tile([64, c_w], f32, tag="C2b")
                    nc.scalar.copy(out=C2b[:], in_=T[64:128, 2, :])
                    if r == 0:
                        # wy = 0.5 exactly: R = T0 + T1 (x0.25 folded later)
                        nc.vector.tensor_tensor(out=R01[:, c_lo:c_lo + c_w],
                                                in0=T[:, 0, :], in1=T[:, 1, :],
                                                op=ADD)
                        nc.vector.tensor_tensor(out=R2[:, c_lo:c_lo + c_w],
                                                in0=T[:64, 2, :], in1=C2b[:],
                                                op=ADD)
                    else:
                        nc.scalar.mul(out=R01[:, c_lo:c_lo + c_w],
                                      in_=T[:, 0, :], mul=wyc0)
                        nc.vector.scalar_tensor_tensor(
                            out=R01[:, c_lo:c_lo + c_w],
                            in0=T[:, 1, :], scalar=wyc1,
                            in1=R01[:, c_lo:c_lo + c_w], op0=MUL, op1=ADD)
                        nc.scalar.mul(out=R2[:, c_lo:c_lo + c_w],
                                      in_=T[:64, 2, :], mul=wyc0[:64])
                        nc.vector.scalar_tensor_tensor(
                            out=R2[:, c_lo:c_lo + c_w],
                            in0=C2b[:], scalar=wyc1[:64],
                            in1=R2[:, c_lo:c_lo + c_w], op0=MUL, op1=ADD)
                return R01, R2

            def body():
                # ---------------- overview (r=0) ----------------
                cx0, w_al = cx0_full, wal_full
                R01, R2 = fetch_and_ycombine(0, cx0, w_al, f32)
                O01 = ovpool.tile([128, CUT], f32, tag="O01")
                O2 = ovpool.tile([64, CUT], f32, tag="O2")
                nc.vector.tensor_tensor(out=O01[:], in0=R01[:, 3::8],
                                        in1=R01[:, 4::8], op=ADD)
                nc.vector.tensor_tensor(out=O2[:], in0=R2[:, 3::8],
                                        in1=R2[:, 4::8], op=ADD)
                O01r = ovpool.tile([128, CUT], f32, tag="O01r")
                O2r = ovpool.tile([64, CUT], f32, tag="O2r")
                nc.vector.tensor_tensor(out=O01r[:], in0=R01[:, 4091::-8],
                                        in1=R01[:, 4092::-8], op=ADD)
                nc.vector.tensor_tensor(out=O2r[:], in0=R2[:, 4091::-8],
                                        in1=R2[:, 4092::-8], op=ADD)
                g, gr = gray_from(O01, O2, scale=0.25)
                nc.vector.tensor_copy(out=gr[:], in_=g[:, ::-1])
                # scale in place (gray already read the unscaled tiles)
                nc.scalar.mul(out=O01[:], in_=O01[:], mul=0.25)
                nc.scalar.mul(out=O2[:], in_=O2[:], mul=0.25)
                nc.scalar.mul(out=O01r[:], in_=O01r[:], mul=0.25)
                nc.scalar.mul(out=O2r[:], in_=O2r[:], mul=0.25)
                odma(out_ap(0, 0, nch=2), O01[:])
                odma(out_ap(0, 2), O2[:])
                for c in range(3):
                    odma(out_ap(1, c), g[:])
                odma(out_ap(2, 0, nch=2), O01r[:])
                odma(out_ap(2, 2), O2r[:])
                for c in range(3):
                    odma(out_ap(3, c), gr[:])

                # ---------------- inner (r=1..12) ----------------
                for ri, (cx0, w_al, coff, blocks) in enumerate(inner_skel):
                    r = ri + 1
                    R01, R2 = fetch_and_ycombine(r, cx0, w_al, bf16)
                    nblk = w_al // 128
                    RT01 = rtpool.tile([128, nblk, 128], bf16, tag="RT01")
                    RT2 = rtpool.tile([128, nblk, 64], bf16, tag="RT2")
                    nc.scalar.dma_start_transpose(RT01[:], R01[:])
                    nc.scalar.dma_start_transpose(RT2[:], R2[:])
                    O01p = ppool.tile([128, CUT], f32, space="PSUM")
                    O2p = ppool.tile([64, CUT], f32, space="PSUM")
                    c = coff
                    for (kb, n_acc, jacc_lo, n_main, jmain_lo) in blocks:
                        if n_acc:
                            rhs = wxb_t[:, c:c + n_acc]
                            nc.tensor.matmul(
                                out=O01p[:, jacc_lo:jacc_lo + n_acc],
                                lhsT=RT01[:, kb, :], rhs=rhs,
                                start=False, stop=True, skip_group_check=True)
                            nc.tensor.matmul(
                                out=O2p[:, jacc_lo:jacc_lo + n_acc],
                                lhsT=RT2[:, kb, :], rhs=rhs,
                                start=False, stop=True, skip_group_check=True)
                            c += n_acc
                        if n_main:
                            rhs = wxb_t[:, c:c + n_main]
                            nc.tensor.matmul(
                                out=O01p[:, jmain_lo:jmain_lo + n_main],
                                lhsT=RT01[:, kb, :], rhs=rhs,
                                start=True, stop=True, skip_group_check=True)
                            nc.tensor.matmul(
                                out=O2p[:, jmain_lo:jmain_lo + n_main],
                                lhsT=RT2[:, kb, :], rhs=rhs,
                                start=True, stop=True, skip_group_check=True)
                            c += n_main
                    O01 = opool.tile([128, CUT], f32, tag="iO01")
                    O2 = opool.tile([64, CUT], f32, tag="iO2")
                    nc.scalar.copy(out=O01[:], in_=O01p[:])
                    nc.vector.tensor_copy(out=O2[:], in_=O2p[:])
                    kout = 3 + r            # inner k -> out[4 + (r-1)]
                    if r == 1:
                        g, _ = gray_from(O01, O2)
                        for cch in range(3):
                            odma(out_ap(kout, cch), g[:])
                    else:
                        odma(out_ap(kout, 0, nch=2), O01[:])
                        odma(out_ap(kout, 2), O2[:])

            if bench:
                with tc.For_i(0, reps) as _i:
                    body()
            else:
                for _rep in range(reps):
                    body()
    return nc


# --------------------------------------------------------------------------
# table construction
# --------------------------------------------------------------------------

def _core_tables(params, core):
    r0 = core * STRIP
    ridx_cols = []
    wy_cols = []
    for p in params:
        y0s = p["y0"][r0:r0 + STRIP].astype(np.int32)
        y1s = p["y1"][r0:r0 + STRIP].astype(np.int32)
        idx = np.zeros(384, np.int32)
        for c2 in range(2):
            idx[c2 * 64:c2 * 64 + 64] = c2 * H + y0s
            idx[128 + c2 * 64:128 + c2 * 64 + 64] = c2 * H + y1s
        idx[256:256 + 64] = 2 * H + y0s
        idx[320:320 + 64] = 2 * H + y1s
        ridx_cols.append(_wrap16(idx))
        wys = p["wy"][r0:r0 + STRIP].astype(np.float32)
        one_m = (np.float32(1.0) - wys).astype(np.float32)
        wy_cols.append(np.stack([np.concatenate([one_m, one_m]),
                                 np.concatenate([wys, wys])], axis=1))
    ridx_all = np.concatenate(ridx_cols, axis=1)                    # [128, 13*24]
    wyt = np.concatenate(wy_cols, axis=1).astype(np.float32)        # [128, 26]
    return ridx_all, wyt


# --------------------------------------------------------------------------
# entry point
# --------------------------------------------------------------------------

def _run(img, specs, trace=False):
    from concourse.bass_utils import run_bass_kernel_spmd

    params = _params(specs)
    skel, wxb = _plan(params)

    if skel in _CACHE:
        nc = _CACHE[skel]
    else:
        nc = _build_bass(skel)
        nc.compile()
        _CACHE[skel] = nc

    in_maps = []
    for core in range(NCORES):
        ridx_all, wyt = _core_tables(params, core)
        in_maps.append({
            "img": img,
            "ridx": ridx_all,
            "wyt": wyt,
            "wxb": wxb,
        })

    r = run_bass_kernel_spmd(nc, in_maps, core_ids=list(range(NCORES)),
                             trace=trace)
    strips = [r.results[c]["out"] for c in range(NCORES)]
    out = np.concatenate(strips, axis=2)
    return out, r


def kernel(**inputs):
    img = np.ascontiguousarray(np.asarray(inputs["input"], np.float32)[0])
    sizes = np.asarray(inputs["sizes"])
    offy = np.asarray(inputs["offy"])
    offx = np.asarray(inputs["offx"])
    specs = _specs_from_inputs(sizes, offy, offx)
    out, _ = _run(img, specs, trace=bool(int(os.environ.get("KERNEL_TRACE", "0"))))
    return out.astype(np.float32)
